# revision 14
# baseline (speedup 1.0000x reference)
"""CrystalGraphConvNet (CGCNN) forward pass on 8 Trainium2 NeuronCores.

Strategy (data-parallel over atoms, feature-major on chip, fp16 compute):
  - 20000 atoms sharded 2500/core, padded to 2560 (= 5 blocks of 512).
  - Activations feature-major in SBUF: x[ot] = [128 chan, 2560 atoms] fp16.
  - Per conv layer:
      * Y' = x @ w1_nbr (atom-major) -> AllGather -> yfull [20480, 512] in DRAM.
      * Neighbor messages fetched with 1024-index dma_gather(transpose=True)
        over 4 SWDGE queues (2 neighbor slots per call), prefetched per block.
      * selfS = x @ w1_self + b1 computed once per 512-atom block (bias folded
        into the PSUM->SBUF copy-out on the DVE).
      * Edge phase runs in weight-batched bursts per (block, m-half, ct):
        6 edge matmuls sharing one w1_edge lhsT load, then 12 identity-inject
        matmuls (yg + selfS) sharing one identity load.  PSUM tiles are
        [128, 2, 512] (two banks) so each holds a neighbor-slot PAIR.
      * relu half (m<6): one fused DVE op per pair:
        gacc = max(psum, 0) + gacc   (scalar_tensor_tensor, 1024 wide).
      * softplus half: Exp(psum) -> Ln(+1) on the scalar engine (both live in
        the single natural_log_exp table; see _gat_combined), 1024 wide,
        then one DVE add into gacc.
      * gacc keeps even/odd slot pairs separate [128, CT, 1024]; folded with
        4 DVE adds before t = gacc @ w2.
      * BN train-mode stats via bn_stats/bn_aggr + tiny AllReduce; rsqrt via
        DVE bit-trick + 2 Newton steps; x' = softplus(x + a*t + b) fused as
        one scalar_tensor_tensor + Exp/Ln, 1024 wide over block pairs.
  - Pooling: each core multiplies its feature-major x by a host-built
    selection matrix Sel [2560, 200] (1/count baked in) via PE transposes +
    matmuls -> partial crystal sums; one small AllReduce combines partials;
    every core runs the tiny head redundantly; core 0's output is used.
Host side only reshapes/shards/remaps indices; all FLOPs are on device.
"""

import os
import numpy as np
import ml_dtypes

import concourse.bacc as bacc
import concourse.bass as bass
import concourse.mybir as mybir
import concourse.tile as tile
from concourse.hw_specs import get_activation_tables as _gat_orig


def _gat_combined(arch):
    # Constrain the act-table-load pass: strip Exp/Ln from every set except
    # the combined natural_log_exp set, so Exp<->Ln never swaps tables.
    tabs = _gat_orig(arch)
    AFt = mybir.ActivationFunctionType
    comb = tabs.get("natural_log_exp_and_others")
    if comb and AFt.Exp in comb and AFt.Ln in comb:
        for nm, s in tabs.items():
            if nm != "natural_log_exp_and_others":
                s.discard(AFt.Exp)
                s.discard(AFt.Ln)
    return tabs


bacc.get_activation_tables = _gat_combined
from concourse.bass_utils import run_bass_kernel_spmd
from concourse.masks import make_identity

F32 = mybir.dt.float32
H16 = mybir.dt.float16
BF16 = mybir.dt.bfloat16
I16 = mybir.dt.int16
I32 = mybir.dt.int32
AF = mybir.ActivationFunctionType
ALU = mybir.AluOpType


class CFG:
    def __init__(self, N=20000, M=12, NBR=64, AFD=256, ORIG=92, H=256, B=200,
                 K=50, N_CONV=3, NC=8, EPS=1e-5):
        self.N, self.M, self.NBR, self.AFD, self.ORIG, self.H = N, M, NBR, AFD, ORIG, H
        self.B, self.K, self.N_CONV, self.NC, self.EPS = B, K, N_CONV, NC, EPS
        assert N % NC == 0
        self.NV = N // NC                    # valid atoms per core
        self.NLP = -(-self.NV // 512) * 512  # padded per-core atoms
        self.ABLK = 512
        self.NAB = self.NLP // self.ABLK     # 512-col tiles per core
        self.NJB = self.NLP // 128           # 128-col blocks per core
        self.GN = NC * self.NLP              # global padded rows
        self.OT = AFD // 128                 # 2 out-feature tiles
        self.CT = 2 * AFD // 128             # 4 hidden tiles
        self.M1 = M // 2
        self.MP = M // 2                     # gather pairs (1024 idx each)
        # bn_stats chunking of the NV valid columns
        nch = 1
        while self.NV // nch > 512 or self.NV % nch:
            nch += 1
        self.BN_NCH, self.BN_W = nch, self.NV // nch


def wrap16(idx, pad_to):
    """int16 index layout for dma_gather: [128, pad_to//16]."""
    a = np.zeros(pad_to, np.int16)
    a[: len(idx)] = idx.astype(np.int16)
    return np.tile(a.reshape(-1, 16).T, (8, 1))


def build_program(cfg: CFG):
    c = cfg
    nc = bacc.Bacc("TRN2", target_bir_lowering=False, debug=False, num_devices=c.NC,
                   num_swdge_queues=4)
    D = {}

    def din(name, shape, dt=F32):
        D[name] = nc.dram_tensor(name, list(shape), dt, kind="ExternalInput")
        return D[name]

    # per-core inputs
    din("atomT", [c.ORIG, c.NLP], H16)                 # embed rhs (zero-padded)
    din("nbrT", [c.NBR, c.M, c.NLP], H16)              # edge features, feature-major
    din("gixp", [128, c.MP * c.NAB * 64], I16)         # paired gather indices
    din("selK", [128, c.NJB * c.B], H16)               # pooling selection k-tiles
    din("m2T", [4, c.B])
    # shared weights
    din("emb_w", [c.ORIG, c.AFD], H16)
    din("emb_b", [128, c.OT])
    din("w1s", [c.N_CONV, 2, 128, 2 * c.AFD], H16)     # lhsT k-tiles
    din("w1n", [c.N_CONV, 2, 128, 2 * c.AFD], H16)     # rhs k-tiles
    din("w1e", [c.N_CONV, c.NBR, 2 * c.AFD], H16)
    din("b1", [128, c.N_CONV * c.CT])
    din("w2", [c.N_CONV, c.CT, 128, c.AFD], H16)
    din("gamma", [128, c.N_CONV * c.OT])
    din("beta", [128, c.N_CONV * c.OT])
    din("fc_w0", [128, c.H])
    din("fc_w1", [128, c.H])
    din("fc_w2", [4, c.H])
    din("fc_b", [128, c.H // 128])
    din("out_w", [128, c.H // 128])
    din("out_b", [1, 1])
    out = nc.dram_tensor("o_out", [c.B], F32, kind="ExternalOutput")
    DBG = int(os.environ.get("DBG", "0"))
    if DBG:
        D["dbg_x"] = nc.dram_tensor("dbg_x", [2, 128, c.NLP], H16, kind="ExternalOutput")
        D["dbg_g"] = nc.dram_tensor("dbg_g", [c.CT, 128, 1024], H16, kind="ExternalOutput")
        D["dbg_t"] = nc.dram_tensor("dbg_t", [2, 128, c.NLP], H16, kind="ExternalOutput")
        D["dbg_xl"] = nc.dram_tensor("dbg_xl", [c.N_CONV, 2, 128, c.NLP], H16, kind="ExternalOutput")
        D["dbg_cr"] = nc.dram_tensor("dbg_cr", [128, 2 * c.B], F32, kind="ExternalOutput")

    NV, NLP, ABLK, NAB, NJB, OT, CT, M, MP = (
        c.NV, c.NLP, c.ABLK, c.NAB, c.NJB, c.OT, c.CT, c.M, c.MP)
    H2 = 2 * c.AFD
    AB2 = 2 * ABLK
    rg = [list(range(c.NC))]

    swdge_ord = [0]

    def swq():
        q = swdge_ord[0] % 4
        swdge_ord[0] += 1
        return q

    with tile.TileContext(nc) as tc:
        with (
            tc.tile_pool(name="persist", bufs=1) as pp,
            tc.tile_pool(name="wts", bufs=1) as wp,
            tc.tile_pool(name="io", bufs=3) as iop,
            tc.tile_pool(name="gat", bufs=16) as gp,
            tc.tile_pool(name="act", bufs=2) as ap_,
            tc.tile_pool(name="selfp", bufs=5) as ssp,
            tc.tile_pool(name="gaccp", bufs=2) as gap,
            tc.tile_pool(name="expp", bufs=3) as ep,
            tc.tile_pool(name="head", bufs=1) as hp,
            tc.tile_pool(name="small", bufs=2) as sp,
            tc.tile_pool(name="psE", bufs=3, space="PSUM") as psE,   # [128,2,512] = 2 banks each
            tc.tile_pool(name="psA", bufs=1, space="PSUM") as psA,   # 1 bank
            tc.tile_pool(name="gsum", bufs=1, space="PSUM") as gs,   # 1 bank
            tc.tile_pool(name="dram", bufs=2, space="DRAM") as dp,
        ):
            # ---- persistent state ----
            x = [pp.tile([128, NLP], H16, tag=f"x{o}", name=f"x{o}") for o in range(OT)]
            t = [pp.tile([128, NLP], H16, tag=f"t{o}", name=f"t{o}") for o in range(OT)]
            ident = pp.tile([128, 128], H16, tag="ident", name="ident")
            make_identity(nc, ident[:])
            gix = pp.tile([128, MP * NAB * 64], I16, tag="gix")
            nc.sync.dma_start(gix[:], D["gixp"][:])
            embb = pp.tile([128, OT], F32, tag="embb", name="embb")
            nc.sync.dma_start(embb[:], D["emb_b"][:])
            b1t = pp.tile([128, c.N_CONV * CT], F32, tag="b1t", name="b1t")
            nc.sync.dma_start(b1t[:], D["b1"][:])
            gmt = pp.tile([128, c.N_CONV * OT], F32, tag="gmt", name="gmt")
            nc.sync.dma_start(gmt[:], D["gamma"][:])
            bet = pp.tile([128, c.N_CONV * OT], F32, tag="bet", name="bet")
            nc.sync.dma_start(bet[:], D["beta"][:])

            # ---- embedding ----
            ew = wp.tile([c.ORIG, c.AFD], H16, tag="ew", name="ew")
            nc.sync.dma_start(ew[:], D["emb_w"][:])
            at = wp.tile([c.ORIG, NLP], H16, tag="at", name="at")
            for ab in range(NAB):
                asl = slice(ab * ABLK, (ab + 1) * ABLK)
                nc.sync.dma_start(at[:, asl], D["atomT"][:, asl])

            if DBG:
                for o in range(OT):
                    nc.sync.dma_start(D["dbg_x"][o], x[o][:])

            # ---- pooling state (filled during the last layer's BN apply) ----
            selt = pp.tile([128, NJB * c.B], H16, tag="selt", name="selt")
            nc.sync.dma_start(selt[:], D["selK"][:])
            crys_acc = pp.tile([128, OT * c.B], F32, tag="crys_acc", name="crys_acc")

            # ---- head weights (loaded early, used at the very end) ----
            m2t = hp.tile([4, c.B], F32, tag="m2t", name="m2t")
            nc.sync.dma_start(m2t[:], D["m2T"][:])
            fw = [hp.tile([128, c.H], F32, tag=f"fw{k}", name=f"fw{k}")
                  for k in range(2)]
            nc.sync.dma_start(fw[0][:], D["fc_w0"][:])
            nc.sync.dma_start(fw[1][:], D["fc_w1"][:])
            fw2 = hp.tile([4, c.H], F32, tag="fw2", name="fw2")
            nc.sync.dma_start(fw2[:], D["fc_w2"][:])
            fbt = hp.tile([128, c.H // 128], F32, tag="fbt", name="fbt")
            nc.sync.dma_start(fbt[:], D["fc_b"][:])
            owt = hp.tile([128, c.H // 128], F32, tag="owt", name="owt")
            nc.sync.dma_start(owt[:], D["out_w"][:])
            obt = hp.tile([1, 1], F32, tag="obt", name="obt")
            nc.sync.dma_start(obt[:], D["out_b"][:])

            # ---- conv layer weights (all layers up front) ----
            w1sT, w1nT, w1eT, w2T = [], [], [], []
            for li in range(c.N_CONV):
                w1st = wp.tile([128, 2 * H2], H16, tag=f"w1s{li}", name=f"w1s{li}")
                w1nt = wp.tile([128, 2 * H2], H16, tag=f"w1n{li}", name=f"w1n{li}")
                for k in range(2):
                    nc.sync.dma_start(w1st[:, k * H2:(k + 1) * H2], D["w1s"][li, k])
                    nc.sync.dma_start(w1nt[:, k * H2:(k + 1) * H2], D["w1n"][li, k])
                w1et = wp.tile([c.NBR, H2], H16, tag=f"w1e{li}", name=f"w1e{li}")
                nc.sync.dma_start(w1et[:], D["w1e"][li])
                w2t = wp.tile([128, CT * c.AFD], H16, tag=f"w2{li}", name=f"w2{li}")
                for k in range(CT):
                    nc.sync.dma_start(w2t[:, k * c.AFD:(k + 1) * c.AFD], D["w2"][li, k])
                w1sT.append(w1st); w1nT.append(w1nt); w1eT.append(w1et); w2T.append(w2t)

            def emit_yjb(w1nt, yloc, jb):
                js = slice(jb * 128, (jb + 1) * 128)
                pt = psA.tile([128, H2], F32, tag="psA", name="yps")
                for k in range(OT):
                    nc.tensor.matmul(pt[:], x[k][:, js],
                                     w1nt[:, k * H2:(k + 1) * H2],
                                     start=(k == 0), stop=(k == OT - 1))
                yb = ap_.tile([128, H2], H16, tag="yb", name="yb")
                nc.vector.tensor_copy(yb[:], pt[:])
                nc.sync.dma_start(yloc[js, :], yb[:])

            def new_ybufs():
                yloc = dp.tile([NLP, H2], H16, tag="yloc", name="yloc")
                yfull = dp.tile([c.GN, H2], H16, tag="yfull", name="yfull",
                                addr_space="Shared")
                return yloc, yfull

            def emit_ag(yloc, yfull):
                nc.gpsimd.collective_compute(
                    "AllGather", ALU.bypass, replica_groups=rg,
                    ins=[yloc.opt()], outs=[yfull.opt()])

            # embed per ab-pair (both o), then emit that pair's Y' immediately
            yloc, yfull = new_ybufs()
            for ab0 in range(0, NAB, 2):
                w = min(2, NAB - ab0)
                wsl = slice(ab0 * ABLK, (ab0 + w) * ABLK)
                for o in range(OT):
                    pt = psE.tile([128, 2, ABLK], F32, tag="psE", name="embps")
                    for h in range(w):
                        sl = slice((ab0 + h) * ABLK, (ab0 + h + 1) * ABLK)
                        nc.tensor.matmul(pt[:, h, :], ew[:, o * 128:(o + 1) * 128],
                                         at[:, sl], start=True, stop=True)
                    e = ep.tile([128, AB2], BF16, tag="e", name="e")
                    nc.scalar.activation(e[:, 0:w * ABLK], pt[:, 0:w, :],
                                         AF.Exp, bias=embb[:, o:o + 1])
                    nc.scalar.activation(x[o][:, wsl], e[:, 0:w * ABLK],
                                         AF.Ln, bias=1.0)
                for jb in range(ab0 * (NJB // NAB), (ab0 + w) * (NJB // NAB)):
                    emit_yjb(w1nT[0], yloc, jb)
            emit_ag(yloc, yfull)

            for li in range(c.N_CONV):
                w1st, w1nt, w1et, w2t = w1sT[li], w1nT[li], w1eT[li], w2T[li]

                # selfS = x@w1s + b1 for ALL blocks (rides the AllGather window)
                selfSs = []
                for ab in range(NAB):
                    sl = slice(ab * ABLK, (ab + 1) * ABLK)
                    selfS = ssp.tile([128, CT, ABLK], H16, tag="selfS",
                                     name=f"selfS{li}_{ab}")
                    for cp in range(CT // 2):
                        pss = psE.tile([128, 2, ABLK], F32, tag="psE", name="selfps")
                        for h in range(2):
                            ct = 2 * cp + h
                            cs = slice(ct * 128, (ct + 1) * 128)
                            for k in range(OT):
                                nc.tensor.matmul(
                                    pss[:, h, :], w1st[:, k * H2:(k + 1) * H2][:, cs],
                                    x[k][:, sl], start=(k == 0), stop=(k == OT - 1))
                        for h in range(2):
                            ct = 2 * cp + h
                            bi = li * CT + ct
                            nc.vector.tensor_scalar(
                                selfS[:, ct, :], pss[:, h, :],
                                b1t[:, bi:bi + 1], None, op0=ALU.add)
                    selfSs.append(selfS)

                for ab in range(NAB):
                    sl = slice(ab * ABLK, (ab + 1) * ABLK)
                    selfS = selfSs[ab]
                    # edge features, one 6-slot half at a time
                    nbh = []
                    for hh_ in range(2):
                        nbt = iop.tile([c.NBR, M // 2, ABLK], H16, tag="nb", name="nb")
                        nc.sync.dma_start(nbt[:], D["nbrT"][:, hh_ * 6:(hh_ + 1) * 6, sl])
                        nbh.append(nbt)
                    # gathers: 6 x 1024 indices, 4 queues, prefetched
                    GMODE = os.environ.get("GMODE", "512")
                    ygs = []          # per m: (tile, col offset)
                    if GMODE == "1024":
                        for mp in range(MP):
                            yg2 = gp.tile([128, CT, AB2], H16, tag="yg", name="yg")
                            nc.gpsimd.dma_gather(
                                yg2[:], yfull[:],
                                gix[:, (mp * NAB + ab) * 64:(mp * NAB + ab + 1) * 64],
                                AB2, AB2, H2, transpose=True,
                                single_packet=False, queue_num=swq())
                            ygs += [(yg2, 0), (yg2, ABLK)]
                    else:
                        for m in range(M):
                            yg1 = gp.tile([128, CT, ABLK], H16, tag="yg1", name="yg1")
                            mp, mi = m // 2, m % 2
                            i0 = (mp * NAB + ab) * 64 + mi * 32
                            nc.gpsimd.dma_gather(
                                yg1[:], yfull[:], gix[:, i0:i0 + 32],
                                ABLK, ABLK, H2, transpose=True,
                                queue_num=swq())
                            ygs += [(yg1, 0)]
                    gacc = gap.tile([128, CT, AB2], H16, tag="gacc", name="gacc")
                    for half in range(2):
                        for ct in range(CT):
                            cs = slice(ct * 128, (ct + 1) * 128)
                            prs = [psE.tile([128, 2, ABLK], F32, tag="psE",
                                            name="pre") for _ in range(3)]
                            # 6 edge matmuls, one w1e lhsT load
                            for p in range(3):
                                mp = half * 3 + p
                                for mi in range(2):
                                    m = 2 * mp + mi
                                    nc.tensor.matmul(prs[p][:, mi, :], w1et[:, cs],
                                                     nbh[half][:, m - half * 6, :],
                                                     start=True, stop=False)
                            # 12 identity injects (yg + selfS), one ident load
                            for p in range(3):
                                mp = half * 3 + p
                                for mi in range(2):
                                    ygt, yo = ygs[2 * mp + mi]
                                    nc.tensor.matmul(prs[p][:, mi, :], ident[:],
                                                     ygt[:, ct, yo:yo + ABLK],
                                                     start=False, stop=False)
                                    nc.tensor.matmul(prs[p][:, mi, :], ident[:],
                                                     selfS[:, ct, :],
                                                     start=False, stop=True)
                            # consumers
                            for p in range(3):
                                mp = half * 3 + p
                                if half == 0:
                                    if mp == 0:
                                        nc.vector.tensor_scalar(
                                            gacc[:, ct, :], prs[p][:, 0:2, :],
                                            0.0, None, op0=ALU.max)
                                    else:
                                        nc.vector.scalar_tensor_tensor(
                                            gacc[:, ct, :], prs[p][:, 0:2, :],
                                            0.0, gacc[:, ct, :],
                                            op0=ALU.max, op1=ALU.add)
                                else:
                                    e = ep.tile([128, AB2], BF16, tag="e", name="e")
                                    nc.scalar.activation(e[:], prs[p][:, 0:2, :],
                                                         AF.Exp)
                                    g = ap_.tile([128, AB2], H16, tag="g", name="g")
                                    nc.scalar.activation(g[:], e[:], AF.Ln, bias=1.0)
                                    nc.vector.tensor_tensor(
                                        gacc[:, ct, :], gacc[:, ct, :], g[:],
                                        op=ALU.add)
                    # fold odd slots into even, then t = gacc @ w2
                    for ct in range(CT):
                        nc.vector.tensor_tensor(
                            gacc[:, ct, 0:ABLK], gacc[:, ct, 0:ABLK],
                            gacc[:, ct, ABLK:AB2], op=ALU.add)
                    if DBG and li == 0 and ab == 0:
                        for ct in range(CT):
                            nc.sync.dma_start(D["dbg_g"][ct], gacc[:, ct, :])
                    ptt = psE.tile([128, 2, ABLK], F32, tag="psE", name="w2ps")
                    for o in range(OT):
                        for ct in range(CT):
                            nc.tensor.matmul(
                                ptt[:, o, :],
                                w2t[:, ct * c.AFD:(ct + 1) * c.AFD][:, o * 128:(o + 1) * 128],
                                gacc[:, ct, 0:ABLK],
                                start=(ct == 0), stop=(ct == CT - 1))
                    for o in range(OT):
                        nc.vector.tensor_copy(t[o][:, sl], ptt[:, o, :])

                if DBG and li == 0:
                    for o in range(OT):
                        nc.sync.dma_start(D["dbg_t"][o], t[o][:])
                # ---- BN stats (valid cols only) + AllReduce ----
                stats = sp.tile([128, 2 * OT], F32, tag="stats", name="stats")
                mv = sp.tile([128, 2 * OT], F32, tag="mv", name="mv")
                st6 = sp.tile([128, c.BN_NCH * 6], F32, tag="st6", name="st6")
                for o in range(OT):
                    for ch in range(c.BN_NCH):
                        nc.vector.bn_stats(
                            st6[:, ch * 6:(ch + 1) * 6],
                            t[o][:, ch * c.BN_W:(ch + 1) * c.BN_W])
                    nc.vector.bn_aggr(mv[:, 2 * o:2 * o + 2], st6[:])
                    # S1 = mean*NV ; S2 = (var + mean^2)*NV
                    sq = sp.tile([128, 1], F32, tag="sq", name="sq")
                    nc.vector.tensor_tensor(sq[:], mv[:, 2 * o:2 * o + 1],
                                            mv[:, 2 * o:2 * o + 1], op=ALU.mult)
                    nc.vector.tensor_tensor(sq[:], sq[:], mv[:, 2 * o + 1:2 * o + 2],
                                            op=ALU.add)
                    nc.vector.tensor_scalar(stats[:, 2 * o + 1:2 * o + 2], sq[:],
                                            float(NV), None, op0=ALU.mult)
                    nc.vector.tensor_scalar(stats[:, 2 * o:2 * o + 1],
                                            mv[:, 2 * o:2 * o + 1],
                                            float(NV), None, op0=ALU.mult)
                bn_i = dp.tile([128, 2 * OT], F32, tag="bn_i", name="bn_i")
                bn_o = dp.tile([128, 2 * OT], F32, tag="bn_o", name="bn_o",
                               addr_space="Shared")
                nc.sync.dma_start(bn_i[:], stats[:])
                nc.gpsimd.collective_compute(
                    "AllReduce", ALU.add, replica_groups=rg,
                    ins=[bn_i.opt()], outs=[bn_o.opt()])
                sg = sp.tile([128, 2 * OT], F32, tag="sg", name="sg")
                nc.sync.dma_start(sg[:], bn_o[:])
                # ---- BN apply + residual + softplus ----
                avecs, bvecs = [], []
                for o in range(OT):
                    lot = li * OT + o
                    mu = sp.tile([128, 1], F32, tag="mu", name="mu")
                    va = sp.tile([128, 1], F32, tag="va", name="va")
                    avec = sp.tile([128, 1], F32, tag="avec", name="avec")
                    bvec = sp.tile([128, 1], F32, tag="bvec", name="bvec")
                    nc.vector.tensor_scalar(mu[:], sg[:, 2 * o:2 * o + 1],
                                            1.0 / c.N, None, op0=ALU.mult)
                    nc.vector.tensor_scalar(va[:], sg[:, 2 * o + 1:2 * o + 2],
                                            1.0 / c.N, None, op0=ALU.mult)
                    nc.vector.tensor_tensor(bvec[:], mu[:], mu[:], op=ALU.mult)
                    nc.vector.tensor_tensor(va[:], va[:], bvec[:], op=ALU.subtract)
                    nc.vector.tensor_scalar(va[:], va[:], float(c.EPS), None,
                                            op0=ALU.add)
                    # rsqrt(va) on DVE: quake seed + 2 Newton steps
                    yq = sp.tile([128, 1], F32, tag="yq", name="yq")
                    yi = yq[:].bitcast(I32)
                    nc.vector.tensor_scalar(yi, va[:].bitcast(I32), 1, None,
                                            op0=ALU.logical_shift_right)
                    nc.vector.tensor_scalar(yi, yi, -1, 0x5f3759df,
                                            op0=ALU.mult, op1=ALU.add)
                    yy = sp.tile([128, 1], F32, tag="yy", name="yy")
                    for _ in range(2):
                        nc.vector.tensor_tensor(yy[:], yq[:], yq[:], op=ALU.mult)
                        nc.vector.tensor_tensor(yy[:], yy[:], va[:], op=ALU.mult)
                        nc.vector.tensor_scalar(yy[:], yy[:], -0.5, 1.5,
                                                op0=ALU.mult, op1=ALU.add)
                        nc.vector.tensor_tensor(yq[:], yq[:], yy[:], op=ALU.mult)
                    nc.vector.tensor_tensor(avec[:], yq[:], gmt[:, lot:lot + 1],
                                            op=ALU.mult)
                    nc.vector.tensor_tensor(bvec[:], mu[:], avec[:], op=ALU.mult)
                    nc.vector.tensor_tensor(bvec[:], bet[:, lot:lot + 1], bvec[:],
                                            op=ALU.subtract)
                    avecs.append(avec)
                    bvecs.append(bvec)
                pool_now = li == c.N_CONV - 1
                more = li + 1 < c.N_CONV
                if more:
                    yloc, yfull = new_ybufs()
                # BN apply over pairs of blocks (1024 wide)
                napply = 0
                for ab0 in range(0, NAB, 2):
                    w = min(2, NAB - ab0)
                    wsl = slice(ab0 * ABLK, (ab0 + w) * ABLK)
                    for o in range(OT):
                        u = ap_.tile([128, AB2], H16, tag="u", name="u")
                        nc.vector.scalar_tensor_tensor(
                            u[:, 0:w * ABLK], t[o][:, wsl], avecs[o][:, 0:1],
                            x[o][:, wsl], op0=ALU.mult, op1=ALU.add)
                        e = ep.tile([128, AB2], BF16, tag="e", name="e")
                        nc.scalar.activation(e[:, 0:w * ABLK], u[:, 0:w * ABLK],
                                             AF.Exp, bias=bvecs[o][:, 0:1])
                        nc.scalar.activation(x[o][:, wsl], e[:, 0:w * ABLK],
                                             AF.Ln, bias=1.0)
                    napply += 1
                    # interleave next layer's Y' / pooling with the BN applies
                    if more:
                        lo = (napply - 1) * (NJB // 3 + 1)
                        hi = min(NJB, napply * (NJB // 3 + 1))
                        for jb in range(lo, hi):
                            emit_yjb(w1nT[li + 1], yloc, jb)
                    if pool_now:
                        jbs = [ab * (NJB // NAB) + jj
                               for ab in range(ab0, ab0 + w)
                               for jj in range(NJB // NAB)]
                        for o in range(OT):
                            cp = gs.tile([128, c.B], F32, tag="gs", name="gsp")
                            for ji, jb in enumerate(jbs):
                                js = slice(jb * 128, (jb + 1) * 128)
                                ptr = psA.tile([128, 128], H16, tag="psA",
                                               name="pst")
                                nc.tensor.transpose(ptr[:], x[o][:, js], ident[:])
                                xT = ap_.tile([128, 128], H16, tag="xT", name="xT")
                                nc.vector.tensor_copy(xT[:], ptr[:])
                                nc.tensor.matmul(
                                    cp[:], xT[:],
                                    selt[:, jb * c.B:(jb + 1) * c.B],
                                    start=(ji == 0), stop=(ji == len(jbs) - 1))
                            osl = slice(o * c.B, (o + 1) * c.B)
                            if ab0 == 0:
                                nc.vector.tensor_copy(crys_acc[:, osl], cp[:])
                            else:
                                nc.vector.tensor_tensor(
                                    crys_acc[:, osl], crys_acc[:, osl], cp[:],
                                    op=ALU.add)
                if DBG:
                    for o in range(OT):
                        nc.sync.dma_start(D["dbg_xl"][li, o], x[o][:])
                if more:
                    emit_ag(yloc, yfull)

            # ---- pooling AllReduce + head (all 200 crystals on every core) ----
            crys_sb = crys_acc
            cr_i = dp.tile([128, OT * c.B], F32, tag="cr_i", name="cr_i")
            cr_o = dp.tile([128, OT * c.B], F32, tag="cr_o", name="cr_o",
                           addr_space="Shared")
            nc.sync.dma_start(cr_i[:], crys_sb[:])
            nc.gpsimd.collective_compute(
                "AllReduce", ALU.add, replica_groups=rg,
                ins=[cr_i.opt()], outs=[cr_o.opt()])
            crysf = hp.tile([128, OT * c.B], F32, tag="crysf", name="crysf")
            nc.sync.dma_start(crysf[:], cr_o[:])
            if DBG:
                nc.sync.dma_start(D["dbg_cr"][:], crysf[:])
            # softplus(crys_cat); head for all 200 crystals on every core
            spc = [hp.tile([128, c.B], F32, tag=f"spc{o}", name=f"spc{o}")
                   for o in range(OT)]
            spm = hp.tile([4, c.B], F32, tag="spm", name="spm")
            for o in range(OT):
                nc.scalar.activation(spc[o][:], crysf[:, o * c.B:(o + 1) * c.B],
                                     AF.Exp)
            nc.scalar.activation(spm[:], m2t[:], AF.Exp)
            for o in range(OT):
                nc.scalar.activation(spc[o][:], spc[o][:], AF.Ln, bias=1.0)
            nc.scalar.activation(spm[:], spm[:], AF.Ln, bias=1.0)
            hh = [hp.tile([128, c.B], F32, tag=f"hh{o}", name=f"hh{o}")
                  for o in range(c.H // 128)]
            for o in range(c.H // 128):
                cs = slice(o * 128, (o + 1) * 128)
                pt = psA.tile([128, ABLK], F32, tag="psA", name="hps")
                nc.tensor.matmul(pt[:, 0:c.B], fw[0][:, cs], spc[0][:],
                                 start=True, stop=False)
                nc.tensor.matmul(pt[:, 0:c.B], fw[1][:, cs], spc[1][:],
                                 start=False, stop=False)
                nc.tensor.matmul(pt[:, 0:c.B], fw2[:, cs], spm[:],
                                 start=False, stop=True)
                nc.scalar.activation(hh[o][:], pt[:, 0:c.B], AF.Exp,
                                     bias=fbt[:, o:o + 1])
                nc.scalar.activation(hh[o][:], hh[o][:], AF.Ln, bias=1.0)
            po = psA.tile([128, ABLK], F32, tag="psA", name="ops")
            for o in range(c.H // 128):
                nc.tensor.matmul(po[0:1, 0:c.B], owt[:, o:o + 1], hh[o][:],
                                 start=(o == 0), stop=(o == c.H // 128 - 1))
            ov = hp.tile([1, c.B], F32, tag="ov", name="ov")
            nc.scalar.activation(ov[:], po[0:1, 0:c.B], AF.Identity,
                                 bias=obt[0:1, 0:1])
            nc.sync.dma_start(out[None, :], ov[:])

    nc.compile()
    return nc


def prep_inputs(inputs, cfg: CFG):
    """Full (unsharded) numpy inputs -> list of 8 per-core input dicts."""
    c = cfg
    f32 = np.float32
    bf = lambda a: np.asarray(a, f32).astype(np.float16)
    atom_fea = np.asarray(inputs["atom_fea"], f32)
    nbr_fea = np.asarray(inputs["nbr_fea"], f32)
    nbr_idx = np.asarray(inputs["nbr_fea_idx"]).astype(np.int64)
    M1 = np.asarray(inputs["M1_index"]).astype(np.int64)
    seg = np.asarray(inputs["seg_ids"]).astype(np.int64)
    m2 = np.asarray(inputs["m2_fea"], f32)
    w1 = np.asarray(inputs["w1"], f32)
    b1 = np.asarray(inputs["b1"], f32)
    w2 = np.asarray(inputs["w2"], f32)
    gam = np.asarray(inputs["gamma"], f32)
    bet = np.asarray(inputs["beta"], f32)

    # shared weight tensors
    shared = {}
    shared["emb_w"] = bf(inputs["emb_w"])
    shared["emb_b"] = np.asarray(inputs["emb_b"], f32).reshape(c.OT, 128).T.copy()
    shared["w1s"] = bf(w1[:, : c.AFD].reshape(c.N_CONV, 2, 128, 2 * c.AFD))
    shared["w1n"] = bf(w1[:, c.AFD:2 * c.AFD].reshape(c.N_CONV, 2, 128, 2 * c.AFD))
    shared["w1e"] = bf(w1[:, 2 * c.AFD:])
    shared["b1"] = np.concatenate(
        [b1[i].reshape(c.CT, 128).T for i in range(c.N_CONV)], 1)
    shared["w2"] = bf(w2.reshape(c.N_CONV, c.CT, 128, c.AFD))
    shared["gamma"] = np.concatenate(
        [gam[i].reshape(c.OT, 128).T for i in range(c.N_CONV)], 1)
    shared["beta"] = np.concatenate(
        [bet[i].reshape(c.OT, 128).T for i in range(c.N_CONV)], 1)
    fc_w = np.asarray(inputs["fc_w"], f32)
    shared["fc_w0"] = fc_w[0:128].copy()
    shared["fc_w1"] = fc_w[128:256].copy()
    shared["fc_w2"] = fc_w[256:260].copy()
    shared["fc_b"] = np.asarray(inputs["fc_b"], f32).reshape(c.H // 128, 128).T.copy()
    shared["out_w"] = np.asarray(inputs["out_w"], f32).reshape(c.H // 128, 128).T.copy()
    shared["out_b"] = np.asarray(inputs["out_b"], f32).reshape(1, 1)
    shared["m2T"] = m2.T.copy()                          # [4, B], same on all cores

    remap = (nbr_idx // c.NV) * c.NLP + (nbr_idx % c.NV)   # -> global padded rows
    counts = np.bincount(seg, minlength=c.B).astype(f32)
    counts[counts == 0] = 1.0
    m1core = M1 // c.NV
    m1loc = M1 % c.NV

    in_maps = []
    for cc in range(c.NC):
        d = dict(shared)
        s = slice(cc * c.NV, (cc + 1) * c.NV)
        atomT = np.zeros((c.ORIG, c.NLP), f32)
        atomT[:, : c.NV] = atom_fea[s].T
        d["atomT"] = bf(atomT)
        nbrT = np.zeros((c.NBR, c.M, c.NLP), f32)
        nbrT[:, :, : c.NV] = nbr_fea[s].transpose(2, 1, 0)
        d["nbrT"] = bf(nbrT)
        # paired gather indices (1024 per call), mp-major, per 512-atom block
        gi = np.zeros((128, c.MP * c.NAB * 64), np.int16)
        rloc = remap[s]                                   # [NV, M]
        cols = np.zeros((c.M, c.NLP), np.int64)
        cols[:, : c.NV] = rloc.T
        for mp in range(c.MP):
            for ab in range(c.NAB):
                asl = slice(ab * c.ABLK, (ab + 1) * c.ABLK)
                pair = np.concatenate([cols[2 * mp, asl], cols[2 * mp + 1, asl]])
                i0 = (mp * c.NAB + ab) * 64
                gi[:, i0:i0 + 64] = wrap16(pair, 2 * c.ABLK)
        d["gixp"] = gi
        # pooling selection for this core's local atoms (1/count baked in)
        sel = np.zeros((c.NLP, c.B), f32)
        own = m1core == cc
        np.add.at(sel, (m1loc[own], seg[own]), 1.0)
        sel /= counts[None, :]
        selK = np.zeros((128, c.NJB * c.B), f32)
        for jb in range(c.NJB):
            selK[:, jb * c.B:(jb + 1) * c.B] = sel[jb * 128:(jb + 1) * 128]
        d["selK"] = bf(selK)
        in_maps.append(d)
    return in_maps


_CACHE = {}


def get_program(cfg=None):
    cfg = cfg or CFG()
    key = tuple(sorted(cfg.__dict__.items()))
    if key not in _CACHE:
        _CACHE[key] = build_program(cfg)
    return _CACHE[key]


def kernel(**inputs):
    cfg = CFG()
    nc = get_program(cfg)
    in_maps = prep_inputs(inputs, cfg)
    res = run_bass_kernel_spmd(nc, in_maps, core_ids=list(range(cfg.NC)))
    out = np.asarray(res.results[0]["o_out"])
    return out.reshape(cfg.B, 1).astype(np.float32)
